# revision 33
# baseline (speedup 1.0000x reference)
"""Bahdanau-attention Trainium2 kernel (Bass/Tile, 8-core data-parallel SPMD).

reference math (per batch b):
    proj_f = features @ W1 + b1                 [T, U]
    proj_h = hidden @ W2 + b2                   [U]
    score  = tanh(proj_f + proj_h)              [T, U]
    logits = score @ V (+ bv, softmax-invariant)[T, 1]
    attn   = softmax(logits, axis=T)            [T, 1]
    context= sum_t attn * features              [D]

Sharding: batch dim B=32 split 4-per-core across 8 cores; weights replicated.

Per-core plan (BL=4, T=2048, D=U=512, P=128), 16 stages of 512 t-columns,
software-pipelined 3 deep on the PE stream, with A-transposes and B-matmuls
interleaved at k-chunk granularity so the DVE psum->sbuf copies are paced:
  A: load F natural [128t,512d], PE-transpose -> FT [128d, t] psum, copy to
     SBUF (DVE, one per stage on ACT for engine balance)
  B: main matmul (float32r, 1 cyc/row): psum[u,t] = sum_k W1[k,uc].T @ FT[k];
     ACT tanh with per-partition bias (hidden@W2 + b1 + b2)[u]
  C: V-dot on PE (logits [1,512] = sum_uc V[uc].T @ score[uc]); per-group max
     m_g (DVE, from PSUM); eager row exp_g = exp(logits - m_g) computed by ACT
     straight from the V-dot PSUM with accumulated sum S_g
  tail (per batch, hidden under next batch's stages): flash-style combine
     (M = max m_g, S = sum e^{m_g-M} S_g), scale exp rows in place by
     e^{m_g-M}/S -> attn row -> DRAM; 16 tiny PE transposes of the attn row
     -> attn_pt [128,16] (f32r); context = 16 accumulated [1,512] matmuls
     lhsT=attn_pt[:,c], rhs=Fn[c].
"""

from contextlib import ExitStack

import numpy as np

import concourse.bass as bass
import concourse.tile as tile
from concourse import bacc, mybir
from concourse.bass_utils import run_bass_kernel_spmd
from concourse.masks import make_identity

f32 = mybir.dt.float32
f32r = mybir.dt.float32r
Tanh = mybir.ActivationFunctionType.Tanh
Exp = mybir.ActivationFunctionType.Exp
AX = mybir.AxisListType.X
MAX = mybir.AluOpType.max
ADD = mybir.AluOpType.add
MULT = mybir.AluOpType.mult

P = 128
B, T, D, U = 32, 2048, 512, 512
NCORES = 8
BL = B // NCORES  # 4 local batches
KC = D // P       # 4 contraction chunks
UC = U // P       # 4 u chunks
TGC = 512         # t columns per group
G = T // TGC      # 4 groups per batch
TT = TGC // P     # 4 t-subtiles per group
NCH = T // P      # 16 t-chunks per batch


def build_kernel(repeat=1, no_ftrans=False, no_ctx=False):
    nc = bacc.Bacc("TRN2", target_bir_lowering=False, debug=False,
                   num_devices=NCORES)

    # features/W1/V feed float32r matmuls; declaring them float32r end-to-end
    # satisfies walrus's "producer must round to FP32r" verification (bytes
    # are plain fp32 either way).
    feat = nc.dram_tensor("features", [BL, T, D], f32r, kind="ExternalInput").ap()
    hid = nc.dram_tensor("hidden", [BL, D], f32, kind="ExternalInput").ap()
    w1 = nc.dram_tensor("W1", [D, U], f32r, kind="ExternalInput").ap()
    b1 = nc.dram_tensor("b1", [U], f32, kind="ExternalInput").ap()
    w2 = nc.dram_tensor("W2", [D, U], f32, kind="ExternalInput").ap()
    b2 = nc.dram_tensor("b2", [U], f32, kind="ExternalInput").ap()
    v = nc.dram_tensor("V", [U, 1], f32r, kind="ExternalInput").ap()
    ctx_out = nc.dram_tensor("context", [BL, D], f32, kind="ExternalOutput").ap()
    attn_out = nc.dram_tensor("attn", [BL, T, 1], f32, kind="ExternalOutput").ap()

    with tile.TileContext(nc) as tc, ExitStack() as ctx:
        consts = ctx.enter_context(tc.tile_pool(name="consts", bufs=1))
        fn_pool = ctx.enter_context(tc.tile_pool(name="fnp", bufs=18))
        ft_pool = ctx.enter_context(tc.tile_pool(name="ftp", bufs=8))
        sc_pool = ctx.enter_context(tc.tile_pool(name="scp", bufs=16))
        row_pool = ctx.enter_context(tc.tile_pool(name="rowp", bufs=4))
        small = ctx.enter_context(tc.tile_pool(name="smallp", bufs=4))
        ps = ctx.enter_context(tc.tile_pool(name="ps", bufs=2, space="PSUM"))

        # ---------------- constants ----------------
        ident = consts.tile([P, P], f32)
        make_identity(nc, ident)
        identr = consts.tile([P, P], f32r)
        nc.vector.tensor_copy(out=identr, in_=ident)

        w1_sb = consts.tile([P, KC, U], f32r)
        nc.sync.dma_start(out=w1_sb, in_=w1.rearrange("(k p) u -> p k u", p=P))
        w2_sb = consts.tile([P, KC, U], f32)
        nc.sync.dma_start(out=w2_sb, in_=w2.rearrange("(k p) u -> p k u", p=P))

        hid_nat = consts.tile([BL, D], f32)
        nc.sync.dma_start(out=hid_nat, in_=hid)

        b1_sb = consts.tile([1, U], f32)
        nc.sync.dma_start(out=b1_sb, in_=b1.rearrange("(o u) -> o u", o=1))
        b2_sb = consts.tile([1, U], f32)
        nc.sync.dma_start(out=b2_sb, in_=b2.rearrange("(o u) -> o u", o=1))
        bsum = consts.tile([1, U], f32)
        nc.vector.tensor_add(bsum, b1_sb, b2_sb)

        v_sb = consts.tile([P, UC], f32r)
        nc.sync.dma_start(out=v_sb, in_=v.rearrange("(k p) o -> p (k o)", p=P))

        ones_bl = consts.tile([1, BL], f32)
        nc.vector.memset(ones_bl, 1.0)

        # hidden transposed: hidT[p, k, b] = hidden[b, k*P + p]
        hidT = consts.tile([P, KC, BL], f32)
        hps = ps.tile([P, KC * BL], f32, tag="vd", bufs=3, name="hps")
        for k in range(KC):
            nc.tensor.transpose(hps[:, k * BL:(k + 1) * BL],
                                hid_nat[:, k * P:(k + 1) * P],
                                ident[0:BL, 0:BL])
        nc.vector.tensor_copy(out=hidT.rearrange("p k b -> p (k b)"), in_=hps)

        # combined per-(u, b) bias: combT[u, b] = (hidden @ W2)[b, u] + b1[u] + b2[u]
        combT = consts.tile([P, UC, BL], f32)
        for uc in range(UC):
            cps = ps.tile([P, BL], f32, tag="vd", bufs=3, name="cps")
            for k in range(KC):
                nc.tensor.matmul(cps, lhsT=w2_sb[:, k, uc * P:(uc + 1) * P],
                                 rhs=hidT[:, k, :],
                                 start=(k == 0), stop=False)
            nc.tensor.matmul(cps, lhsT=bsum[0:1, uc * P:(uc + 1) * P],
                             rhs=ones_bl, start=False, stop=True)
            nc.vector.tensor_copy(out=combT[:, uc, :], in_=cps)

        # ---------------- per-batch state ----------------
        fn_tiles = {}
        exps = {}      # b -> exp(l - m_g) row [1, T]; scaled in place at tail
        m4 = {}        # b -> per-group max [1, G]
        nm4 = {}       # b -> negated per-group max [1, G]
        s4 = {}        # b -> per-group exp-sum [1, G]

        def load_fn(b):
            # double-width loads: one DMA covers two 128-row chunks
            for chp in range(NCH // 2):
                fnt2 = fn_pool.tile([P, 2, D], f32r, tag="fn", name="fnt2")
                nc.sync.dma_start(
                    out=fnt2,
                    in_=feat[b, chp * 2 * P:(chp + 1) * 2 * P, :].rearrange(
                        "(c p) d -> p c d", p=P))
                fn_tiles[(b, 2 * chp)] = fnt2[:, 0, :]
                fn_tiles[(b, 2 * chp + 1)] = fnt2[:, 1, :]

        def alloc_batch(b):
            exps[b] = row_pool.tile([1, T], f32, tag="erow", bufs=2, name="exps_row")
            m4[b] = small.tile([1, G], f32, tag="m4", name="m4")
            nm4[b] = small.tile([1, G], f32, tag="nm4", name="nm4")
            s4[b] = small.tile([1, G], f32, tag="s4", name="s4")

        def stage_A_k(b, g, k):
            """F transpose for one (group, k-chunk) -> FT sbuf tile."""
            if no_ftrans:
                return fn_tiles[(b, g * TT + (k % TT))]
            ftp = ps.tile([P, TGC], f32r, tag="ft", bufs=3, name="ftp")
            for tt in range(TT):
                fnt = fn_tiles[(b, g * TT + tt)]
                nc.tensor.transpose(ftp[:, tt * P:(tt + 1) * P],
                                    fnt[:, k * P:(k + 1) * P], identr)
            ft_sb = ft_pool.tile([P, TGC], f32r, tag="ft", name="ft_sb")
            if k != 3:
                nc.vector.tensor_copy(out=ft_sb, in_=ftp)
            else:
                nc.scalar.copy(out=ft_sb, in_=ftp)
            return ft_sb

        def stage_B_u(b, g, fts, uc):
            """Main matmul + tanh for one (group, u-chunk) -> score tile."""
            mmp = ps.tile([P, TGC], f32, tag="mm", bufs=2, name="mmp")
            for k in range(KC):
                nc.tensor.matmul(mmp,
                                 lhsT=w1_sb[:, k, uc * P:(uc + 1) * P],
                                 rhs=fts[k],
                                 start=(k == 0), stop=(k == KC - 1))
            sct = sc_pool.tile([P, TGC], f32r, tag="sc", name="sct")
            nc.scalar.activation(out=sct, in_=mmp, func=Tanh,
                                 bias=combT[:, uc, b:b + 1], scale=1.0)
            return sct

        def stage_C(b, g, scs):
            """V-dot, per-group softmax partials, logits chunk layout."""
            gsl = slice(g * TGC, (g + 1) * TGC)
            vp = ps.tile([1, TGC], f32, tag="vd", bufs=3, name="vp")
            for uc in range(UC):
                nc.tensor.matmul(vp, lhsT=v_sb[:, uc:uc + 1], rhs=scs[uc],
                                 start=(uc == 0), stop=(uc == UC - 1))
            # group max + eager exp row straight from the V-dot PSUM
            nc.vector.tensor_reduce(m4[b][0:1, g:g + 1], vp, axis=AX, op=MAX)
            nc.vector.tensor_scalar_mul(nm4[b][0:1, g:g + 1],
                                        m4[b][0:1, g:g + 1], -1.0)
            nc.scalar.activation(out=exps[b][0:1, gsl], in_=vp,
                                 func=Exp, bias=nm4[b][0:1, g:g + 1],
                                 scale=1.0,
                                 accum_out=s4[b][0:1, g:g + 1])

        def tail(b):
            """Combine group partials, attn outputs, context."""
            # global max and 1/S
            gm = small.tile([1, 1], f32, tag="g1", name="gm")
            nc.vector.tensor_reduce(gm, m4[b], axis=AX, op=MAX)
            ngm = small.tile([1, 1], f32, tag="g1", name="ngm")
            nc.vector.tensor_scalar_mul(ngm, gm, -1.0)
            w4 = small.tile([1, G], f32, tag="w4", name="w4")
            nc.scalar.activation(out=w4, in_=m4[b], func=Exp, bias=ngm,
                                 scale=1.0)
            ws = small.tile([1, G], f32, tag="w4", name="ws")
            nc.vector.tensor_mul(ws, w4, s4[b])
            S = small.tile([1, 1], f32, tag="g1", name="S")
            nc.vector.tensor_reduce(S, ws, axis=AX, op=ADD)
            rs = small.tile([1, 1], f32, tag="g1", name="rs")
            nc.vector.reciprocal(rs, S)
            # attn row: scale the eager exp row in place by w_g / S per group
            f4 = small.tile([1, G], f32, tag="w4", name="f4")
            nc.vector.tensor_scalar_mul(f4, w4, rs)
            for g in range(G):
                gsl = slice(g * TGC, (g + 1) * TGC)
                nc.vector.tensor_scalar_mul(exps[b][0:1, gsl],
                                            exps[b][0:1, gsl],
                                            f4[0:1, g:g + 1])
            nc.sync.dma_start(
                out=attn_out[b].rearrange("(o t) one -> o (t one)", o=1),
                in_=exps[b])
            # attn in chunk layout: tiny PE transposes of the scaled row
            atp = ps.tile([P, NCH], f32, tag="vd", bufs=3, name="atp")
            for c in range(NCH):
                nc.tensor.transpose(atp[:, c:c + 1],
                                    exps[b][0:1, c * P:(c + 1) * P],
                                    ident[0:1, 0:1])
            attn_pt = small.tile([P, NCH], f32r, tag="apt", name="attn_pt")
            nc.vector.tensor_copy(out=attn_pt, in_=atp)
            # context accumulation over the 16 chunks
            ctx_sb = small.tile([1, D], f32, tag="cxs", name="ctx_sb")
            if no_ctx:
                nc.vector.memset(ctx_sb, 0.0)
            else:
                cxp = ps.tile([1, D], f32, tag="vd", bufs=3, name="cxp")
                for c in range(NCH):
                    nc.tensor.matmul(cxp, lhsT=attn_pt[:, c:c + 1],
                                     rhs=fn_tiles[(b, c)],
                                     start=(c == 0), stop=(c == NCH - 1))
                nc.vector.tensor_copy(out=ctx_sb, in_=cxp)
            nc.sync.dma_start(out=ctx_out[b:b + 1, :], in_=ctx_sb)

        # ---------------- pipelined emission ----------------
        # `repeat` reruns the whole pipeline inside one NEFF; used by the
        # timing harness to measure marginal per-iteration time. The graded
        # kernel uses repeat=1.
        stages = [(r, b, g) for r in range(repeat)
                  for b in range(BL) for g in range(G)]
        n = len(stages)
        load_fn(0)
        ftq = {}
        scq = {}

        def emit_C(i):
            r, b, g = stages[i]
            stage_C(b, g, scq.pop(i))
            if g == G - 1:
                tail(b)

        for i, (r, b, g) in enumerate(stages):
            if g == 0:
                alloc_batch(b)
            fts_i = []
            scs_p = []
            for k in range(KC):
                fts_i.append(stage_A_k(b, g, k))
                if i >= 1:
                    pr, pb, pg = stages[i - 1]
                    scs_p.append(stage_B_u(pb, pg, ftq[i - 1], k))

            ftq[i] = fts_i
            if i >= 1:
                del ftq[i - 1]
                scq[i - 1] = scs_p
            if i >= 2:
                emit_C(i - 2)
            if g == 1 and (b + 1 < BL or r + 1 < repeat):
                load_fn((b + 1) % BL)
        pr, pb, pg = stages[n - 1]
        fts_last = ftq.pop(n - 1)
        scq[n - 1] = [stage_B_u(pb, pg, fts_last, uc) for uc in range(UC)]
        emit_C(n - 2)
        emit_C(n - 1)

    nc.compile()
    return nc


_NC_CACHE = None


def _get_nc():
    global _NC_CACHE
    if _NC_CACHE is None:
        _NC_CACHE = build_kernel()
    return _NC_CACHE


def _as_np(x):
    return np.ascontiguousarray(np.asarray(x, dtype=np.float32))


def make_in_maps(features, hidden, W1, b1, W2, b2, V):
    features = _as_np(features)
    hidden = _as_np(hidden)
    W1, b1, W2, b2, V = map(_as_np, (W1, b1, W2, b2, V))
    in_maps = []
    for c in range(NCORES):
        sl = slice(c * BL, (c + 1) * BL)
        in_maps.append({
            "features": features[sl],
            "hidden": hidden[sl],
            "W1": W1, "b1": b1, "W2": W2, "b2": b2, "V": V,
        })
    return in_maps


def kernel(features, hidden, W1, b1, W2, b2, V, bv):
    # bv shifts logits by a constant; softmax is shift-invariant and bv does
    # not appear in either output, so it is unused.
    nc = _get_nc()
    in_maps = make_in_maps(features, hidden, W1, b1, W2, b2, V)
    res = run_bass_kernel_spmd(nc, in_maps, core_ids=list(range(NCORES)))
    context = np.concatenate([res.results[c]["context"] for c in range(NCORES)],
                             axis=0)
    attn = np.concatenate([res.results[c]["attn"] for c in range(NCORES)],
                          axis=0)
    return context, attn


# revision 34
# speedup vs baseline: 1.5377x; 1.5377x over previous
"""Bahdanau-attention Trainium2 kernel (Bass/Tile, 8-core data-parallel SPMD).

reference math (per batch b):
    proj_f = features @ W1 + b1                 [T, U]
    proj_h = hidden @ W2 + b2                   [U]
    score  = tanh(proj_f + proj_h)              [T, U]
    logits = score @ V (+ bv, softmax-invariant)[T, 1]
    attn   = softmax(logits, axis=T)            [T, 1]
    context= sum_t attn * features              [D]

Sharding: batch dim B=32 split 4-per-core across 8 cores; weights replicated.

Per-core plan (BL=4, T=2048, D=U=512, P=128), 16 stages of 512 t-columns,
software-pipelined 3 deep on the PE stream, with A-transposes and B-matmuls
interleaved at k-chunk granularity so the DVE psum->sbuf copies are paced:
  A: load F natural [128t,512d], PE-transpose -> FT [128d, t] psum, copy to
     SBUF (DVE, one per stage on ACT for engine balance)
  B: main matmul (float32r, 1 cyc/row): psum[u,t] = sum_k W1[k,uc].T @ FT[k];
     ACT tanh with per-partition bias (hidden@W2 + b1 + b2)[u]
  C: V-dot on PE (logits [1,512] = sum_uc V[uc].T @ score[uc]); per-group max
     m_g (DVE, from PSUM); eager row exp_g = exp(logits - m_g) computed by ACT
     straight from the V-dot PSUM with accumulated sum S_g
  tail (per batch, hidden under next batch's stages): flash-style combine
     (M = max m_g, S = sum e^{m_g-M} S_g), scale exp rows in place by
     e^{m_g-M}/S -> attn row -> DRAM; 16 tiny PE transposes of the attn row
     -> attn_pt [128,16] (f32r); context = 16 accumulated [1,512] matmuls
     lhsT=attn_pt[:,c], rhs=Fn[c].
"""

from contextlib import ExitStack

import numpy as np

import concourse.bass as bass
import concourse.tile as tile
from concourse import bacc, mybir
from concourse.bass_utils import run_bass_kernel_spmd
from concourse.masks import make_identity

f32 = mybir.dt.float32
f32r = mybir.dt.float32r
Tanh = mybir.ActivationFunctionType.Tanh
Exp = mybir.ActivationFunctionType.Exp
AX = mybir.AxisListType.X
MAX = mybir.AluOpType.max
ADD = mybir.AluOpType.add
MULT = mybir.AluOpType.mult

P = 128
B, T, D, U = 32, 2048, 512, 512
NCORES = 8
BL = B // NCORES  # 4 local batches
KC = D // P       # 4 contraction chunks
UC = U // P       # 4 u chunks
TGC = 512         # t columns per group
G = T // TGC      # 4 groups per batch
TT = TGC // P     # 4 t-subtiles per group
NCH = T // P      # 16 t-chunks per batch


def build_kernel(repeat=1, no_ftrans=False, no_ctx=False):
    nc = bacc.Bacc("TRN2", target_bir_lowering=False, debug=False,
                   num_devices=NCORES)

    # features/W1/V feed float32r matmuls; declaring them float32r end-to-end
    # satisfies walrus's "producer must round to FP32r" verification (bytes
    # are plain fp32 either way).
    feat = nc.dram_tensor("features", [BL, T, D], f32r, kind="ExternalInput").ap()
    hid = nc.dram_tensor("hidden", [BL, D], f32, kind="ExternalInput").ap()
    w1 = nc.dram_tensor("W1", [D, U], f32r, kind="ExternalInput").ap()
    b1 = nc.dram_tensor("b1", [U], f32, kind="ExternalInput").ap()
    w2 = nc.dram_tensor("W2", [D, U], f32, kind="ExternalInput").ap()
    b2 = nc.dram_tensor("b2", [U], f32, kind="ExternalInput").ap()
    v = nc.dram_tensor("V", [U, 1], f32r, kind="ExternalInput").ap()
    ctx_out = nc.dram_tensor("context", [BL, D], f32, kind="ExternalOutput").ap()
    attn_out = nc.dram_tensor("attn", [BL, T, 1], f32, kind="ExternalOutput").ap()

    with tile.TileContext(nc) as tc, ExitStack() as ctx:
        consts = ctx.enter_context(tc.tile_pool(name="consts", bufs=1))
        fn_pool = ctx.enter_context(tc.tile_pool(name="fnp", bufs=18))
        ft_pool = ctx.enter_context(tc.tile_pool(name="ftp", bufs=8))
        sc_pool = ctx.enter_context(tc.tile_pool(name="scp", bufs=16))
        row_pool = ctx.enter_context(tc.tile_pool(name="rowp", bufs=4))
        small = ctx.enter_context(tc.tile_pool(name="smallp", bufs=4))
        ps = ctx.enter_context(tc.tile_pool(name="ps", bufs=2, space="PSUM"))

        # ---------------- constants ----------------
        ident = consts.tile([P, P], f32)
        make_identity(nc, ident)
        identr = consts.tile([P, P], f32r)
        nc.vector.tensor_copy(out=identr, in_=ident)

        w1_sb = consts.tile([P, KC, U], f32r)
        nc.sync.dma_start(out=w1_sb, in_=w1.rearrange("(k p) u -> p k u", p=P))
        w2_sb = consts.tile([P, KC, U], f32)
        nc.sync.dma_start(out=w2_sb, in_=w2.rearrange("(k p) u -> p k u", p=P))

        hid_nat = consts.tile([BL, D], f32)
        nc.sync.dma_start(out=hid_nat, in_=hid)

        b1_sb = consts.tile([1, U], f32)
        nc.sync.dma_start(out=b1_sb, in_=b1.rearrange("(o u) -> o u", o=1))
        b2_sb = consts.tile([1, U], f32)
        nc.sync.dma_start(out=b2_sb, in_=b2.rearrange("(o u) -> o u", o=1))
        bsum = consts.tile([1, U], f32)
        nc.vector.tensor_add(bsum, b1_sb, b2_sb)

        v_sb = consts.tile([P, UC], f32r)
        nc.sync.dma_start(out=v_sb, in_=v.rearrange("(k p) o -> p (k o)", p=P))

        ones_bl = consts.tile([1, BL], f32)
        nc.vector.memset(ones_bl, 1.0)

        # hidden transposed: hidT[p, k, b] = hidden[b, k*P + p]
        hidT = consts.tile([P, KC, BL], f32)
        hps = ps.tile([P, KC * BL], f32, tag="vd", bufs=3, name="hps")
        for k in range(KC):
            nc.tensor.transpose(hps[:, k * BL:(k + 1) * BL],
                                hid_nat[:, k * P:(k + 1) * P],
                                ident[0:BL, 0:BL])
        nc.vector.tensor_copy(out=hidT.rearrange("p k b -> p (k b)"), in_=hps)

        # combined per-(u, b) bias: combT[u, b] = (hidden @ W2)[b, u] + b1[u] + b2[u]
        combT = consts.tile([P, UC, BL], f32)
        for uc in range(UC):
            cps = ps.tile([P, BL], f32, tag="vd", bufs=3, name="cps")
            for k in range(KC):
                nc.tensor.matmul(cps, lhsT=w2_sb[:, k, uc * P:(uc + 1) * P],
                                 rhs=hidT[:, k, :],
                                 start=(k == 0), stop=False)
            nc.tensor.matmul(cps, lhsT=bsum[0:1, uc * P:(uc + 1) * P],
                             rhs=ones_bl, start=False, stop=True)
            nc.vector.tensor_copy(out=combT[:, uc, :], in_=cps)

        # ---------------- per-batch state ----------------
        fn_tiles = {}
        exps = {}      # b -> exp(l - m_g) row [1, T]; scaled in place at tail
        m4 = {}        # b -> per-group max [1, G]
        nm4 = {}       # b -> negated per-group max [1, G]
        s4 = {}        # b -> per-group exp-sum [1, G]

        def load_fn(b):
            # double-width loads: one DMA covers two 128-row chunks
            for chp in range(NCH // 2):
                fnt2 = fn_pool.tile([P, 2, D], f32r, tag="fn", name="fnt2")
                nc.sync.dma_start(
                    out=fnt2,
                    in_=feat[b, chp * 2 * P:(chp + 1) * 2 * P, :].rearrange(
                        "(c p) d -> p c d", p=P))
                fn_tiles[(b, 2 * chp)] = fnt2[:, 0, :]
                fn_tiles[(b, 2 * chp + 1)] = fnt2[:, 1, :]

        def alloc_batch(b):
            exps[b] = row_pool.tile([1, T], f32, tag="erow", bufs=2, name="exps_row")
            m4[b] = small.tile([1, G], f32, tag="m4", name="m4")
            nm4[b] = small.tile([1, G], f32, tag="nm4", name="nm4")
            s4[b] = small.tile([1, G], f32, tag="s4", name="s4")

        def stage_A_k(b, g, k):
            """F transpose for one (group, k-chunk) -> FT sbuf tile."""
            if no_ftrans:
                return fn_tiles[(b, g * TT + (k % TT))]
            ftp = ps.tile([P, TGC], f32r, tag="ft", bufs=3, name="ftp")
            for tt in range(TT):
                fnt = fn_tiles[(b, g * TT + tt)]
                nc.tensor.transpose(ftp[:, tt * P:(tt + 1) * P],
                                    fnt[:, k * P:(k + 1) * P], identr)
            ft_sb = ft_pool.tile([P, TGC], f32r, tag="ft", name="ft_sb")
            if k != 3:
                nc.vector.tensor_copy(out=ft_sb, in_=ftp)
            else:
                nc.scalar.copy(out=ft_sb, in_=ftp)
            return ft_sb

        def stage_B_u(b, g, fts, uc):
            """Main matmul + tanh for one (group, u-chunk) -> score tile."""
            mmp = ps.tile([P, TGC], f32, tag="mm", bufs=2, name="mmp")
            for k in range(KC):
                nc.tensor.matmul(mmp,
                                 lhsT=w1_sb[:, k, uc * P:(uc + 1) * P],
                                 rhs=fts[k],
                                 start=(k == 0), stop=(k == KC - 1))
            sct = sc_pool.tile([P, TGC], f32r, tag="sc", name="sct")
            nc.scalar.activation(out=sct, in_=mmp, func=Tanh,
                                 bias=combT[:, uc, b:b + 1], scale=1.0)
            return sct

        def stage_C(b, g, scs):
            """V-dot, per-group softmax partials, logits chunk layout."""
            gsl = slice(g * TGC, (g + 1) * TGC)
            vp = ps.tile([1, TGC], f32, tag="vd", bufs=3, name="vp")
            for uc in range(UC):
                nc.tensor.matmul(vp, lhsT=v_sb[:, uc:uc + 1], rhs=scs[uc],
                                 start=(uc == 0), stop=(uc == UC - 1))
            # group max + eager exp row straight from the V-dot PSUM
            nc.vector.tensor_reduce(m4[b][0:1, g:g + 1], vp, axis=AX, op=MAX)
            nc.vector.tensor_scalar_mul(nm4[b][0:1, g:g + 1],
                                        m4[b][0:1, g:g + 1], -1.0)
            nc.scalar.activation(out=exps[b][0:1, gsl], in_=vp,
                                 func=Exp, bias=nm4[b][0:1, g:g + 1],
                                 scale=1.0,
                                 accum_out=s4[b][0:1, g:g + 1])

        def tail(b):
            """Combine group partials, attn outputs, context."""
            # global max and 1/S
            gm = small.tile([1, 1], f32, tag="g1", name="gm")
            nc.vector.tensor_reduce(gm, m4[b], axis=AX, op=MAX)
            ngm = small.tile([1, 1], f32, tag="g1", name="ngm")
            nc.vector.tensor_scalar_mul(ngm, gm, -1.0)
            w4 = small.tile([1, G], f32, tag="w4", name="w4")
            nc.scalar.activation(out=w4, in_=m4[b], func=Exp, bias=ngm,
                                 scale=1.0)
            ws = small.tile([1, G], f32, tag="w4", name="ws")
            nc.vector.tensor_mul(ws, w4, s4[b])
            S = small.tile([1, 1], f32, tag="g1", name="S")
            nc.vector.tensor_reduce(S, ws, axis=AX, op=ADD)
            rs = small.tile([1, 1], f32, tag="g1", name="rs")
            nc.vector.reciprocal(rs, S)
            # attn row: scale the eager exp row in place by w_g / S per group
            f4 = small.tile([1, G], f32, tag="w4", name="f4")
            nc.vector.tensor_scalar_mul(f4, w4, rs)
            for g in range(G):
                gsl = slice(g * TGC, (g + 1) * TGC)
                nc.vector.tensor_scalar_mul(exps[b][0:1, gsl],
                                            exps[b][0:1, gsl],
                                            f4[0:1, g:g + 1])
            nc.sync.dma_start(
                out=attn_out[b].rearrange("(o t) one -> o (t one)", o=1),
                in_=exps[b])
            # attn in chunk layout: tiny PE transposes of the scaled row
            atp = ps.tile([P, NCH], f32, tag="vd", bufs=3, name="atp")
            for c in range(NCH):
                nc.tensor.transpose(atp[:, c:c + 1],
                                    exps[b][0:1, c * P:(c + 1) * P],
                                    ident[0:1, 0:1])
            attn_pt = small.tile([P, NCH], f32r, tag="apt", name="attn_pt")
            nc.vector.tensor_copy(out=attn_pt, in_=atp)
            # context accumulation over the 16 chunks
            ctx_sb = small.tile([1, D], f32, tag="cxs", name="ctx_sb")
            if no_ctx:
                nc.vector.memset(ctx_sb, 0.0)
            else:
                cxp = ps.tile([1, D], f32, tag="vd", bufs=3, name="cxp")
                for c in range(NCH):
                    nc.tensor.matmul(cxp, lhsT=attn_pt[:, c:c + 1],
                                     rhs=fn_tiles[(b, c)],
                                     start=(c == 0), stop=(c == NCH - 1))
                nc.vector.tensor_copy(out=ctx_sb, in_=cxp)
            nc.sync.dma_start(out=ctx_out[b:b + 1, :], in_=ctx_sb)

        # ---------------- pipelined emission ----------------
        # `repeat` reruns the whole pipeline inside one NEFF; used by the
        # timing harness to measure marginal per-iteration time. The graded
        # kernel uses repeat=1.
        stages = [(r, b, g) for r in range(repeat)
                  for b in range(BL) for g in range(G)]
        n = len(stages)
        load_fn(0)
        ftq = {}
        scq = {}

        def emit_C(i):
            r, b, g = stages[i]
            stage_C(b, g, scq.pop(i))
            if g == G - 1:
                tail(b)

        for i, (r, b, g) in enumerate(stages):
            if g == 0:
                alloc_batch(b)
            fts_i = []
            scs_p = []
            for k in range(KC):
                fts_i.append(stage_A_k(b, g, k))
                if k % 2 == 1 and i >= 1:
                    pr, pb, pg = stages[i - 1]
                    scs_p.append(stage_B_u(pb, pg, ftq[i - 1], k - 1))
                    scs_p.append(stage_B_u(pb, pg, ftq[i - 1], k))

            ftq[i] = fts_i
            if i >= 1:
                del ftq[i - 1]
                scq[i - 1] = scs_p
            if i >= 2:
                emit_C(i - 2)
            if g == 1 and (b + 1 < BL or r + 1 < repeat):
                load_fn((b + 1) % BL)
        pr, pb, pg = stages[n - 1]
        fts_last = ftq.pop(n - 1)
        scq[n - 1] = [stage_B_u(pb, pg, fts_last, uc) for uc in range(UC)]
        emit_C(n - 2)
        emit_C(n - 1)

    nc.compile()
    return nc


_NC_CACHE = None


def _get_nc():
    global _NC_CACHE
    if _NC_CACHE is None:
        _NC_CACHE = build_kernel()
    return _NC_CACHE


def _as_np(x):
    return np.ascontiguousarray(np.asarray(x, dtype=np.float32))


def make_in_maps(features, hidden, W1, b1, W2, b2, V):
    features = _as_np(features)
    hidden = _as_np(hidden)
    W1, b1, W2, b2, V = map(_as_np, (W1, b1, W2, b2, V))
    in_maps = []
    for c in range(NCORES):
        sl = slice(c * BL, (c + 1) * BL)
        in_maps.append({
            "features": features[sl],
            "hidden": hidden[sl],
            "W1": W1, "b1": b1, "W2": W2, "b2": b2, "V": V,
        })
    return in_maps


def kernel(features, hidden, W1, b1, W2, b2, V, bv):
    # bv shifts logits by a constant; softmax is shift-invariant and bv does
    # not appear in either output, so it is unused.
    nc = _get_nc()
    in_maps = make_in_maps(features, hidden, W1, b1, W2, b2, V)
    res = run_bass_kernel_spmd(nc, in_maps, core_ids=list(range(NCORES)))
    context = np.concatenate([res.results[c]["context"] for c in range(NCORES)],
                             axis=0)
    attn = np.concatenate([res.results[c]["attn"] for c in range(NCORES)],
                          axis=0)
    return context, attn
